# revision 61
# baseline (speedup 1.0000x reference)
"""GQA attention (B=2,T=2048,C=2048,H=16,KV=4,D=128) + RoPE + causal softmax
+ output projection, tensor-parallel over 8 NeuronCores (2 q-heads/core).

Contract: kernel(**inputs) takes full numpy inputs, returns full output.
Per-core partial outputs (o @ Wo[rows]) are summed on the host.
"""

import sys

sys.path.insert(0, "/opt/trn_rl_repo")

import numpy as np

import concourse.bacc as bacc
import concourse.bass as bass
import concourse.mybir as mybir
import concourse.tile as tile
from concourse.tile import add_dep_helper
from concourse.bass_utils import run_bass_kernel_spmd

B, T, C = 2, 2048, 2048
H, KV, D = 16, 4, 128
G = H // KV
N_CORES = 8
HL = H // N_CORES  # 2 q-heads per core
BT = B * T  # 4096
NCH = BT // 512  # 8 token chunks of 512
KCH = C // 128  # 16 contraction chunks
QC = T // 512  # 4 q chunks per batch
KC = T // 128  # 16 k chunks per batch

F16 = mybir.dt.float16
F32 = mybir.dt.float32

_cache = {}


class _St:
    pass


SPLIT_MM = False
USE_CC = True
DEBUG_KV = False


def _mm2(nc, out, lhsT, rhs, start, stop):
    """M=128 matmul, optionally as two col-tiled M=64 halves (the second
    LDWEIGHTS can overlap the first matmul in the other column group)."""
    if not SPLIT_MM:
        nc.tensor.matmul(out[:, :], lhsT=lhsT, rhs=rhs,
                         start=start, stop=stop)
        return
    nc.tensor.matmul(out[0:64, :], lhsT=lhsT[:, 0:64], rhs=rhs,
                     start=start, stop=stop)
    nc.tensor.matmul(out[64:128, :], lhsT=lhsT[:, 64:128], rhs=rhs,
                     start=start, stop=stop, tile_position=(0, 64))


def _emit_p1_chunk(nc, st, n, with_kv=True):
    """QKV projections for one 512-token chunk.

    K/V are only computed for the core's own batch half (chunks 0..3);
    the partner core's half arrives via a 2-core AllGather."""
    mult = mybir.AluOpType.mult

    def rope(dst, ps, cos_s, sin_s, t0):
        t1 = st.rpool.tile([128, 512], F32, name="t1")
        t2 = st.rpool.tile([128, 512], F32, name="t2")
        nc.vector.tensor_tensor(t1[:], ps[:], cos_s[:, t0:t0 + 512], mult)
        nc.vector.tensor_tensor(t2[0:64, :], ps[64:128, :],
                                sin_s[0:64, t0:t0 + 512], mult)
        nc.vector.tensor_tensor(t2[64:128, :], ps[0:64, :],
                                sin_s[64:128, t0:t0 + 512], mult)
        nc.vector.tensor_tensor(dst, t1[:], t2[:], mybir.AluOpType.add)

    n0 = n * 512
    t0 = (n % QC) * 512  # rope-table offset (within batch)
    xt = st.xpool.tile([128, KCH, 512], F16, name="xt")
    dma_eng = nc.sync if n % 2 == 0 else nc.scalar
    dma_eng.dma_start(xt[:], st.xTp[n])
    for h in range(HL):
        psq = st.psA.tile([128, 512], F32, name="psq", tag="psA")
        for kc in range(KCH):
            _mm2(nc, psq, st.wq_s[:, kc, h * D:(h + 1) * D],
                 xt[:, kc, :], (kc == 0), (kc == KCH - 1))
        rope(st.qT[:, h, n0:n0 + 512], psq, st.cosq_s, st.sinq_s, t0)
    if not with_kv:
        return
    psk = st.psA.tile([128, 512], F32, name="psk", tag="psA")
    for kc in range(KCH):
        _mm2(nc, psk, st.wk_s[:, kc, :], xt[:, kc, :],
             (kc == 0), (kc == KCH - 1))
    rope(st.kT[:, n0:n0 + 512], psk, st.cosk_s, st.sink_s, t0)
    # v computed in transposed layout (N=512 matmuls), then moved to the
    # natural [token, d] layout via the DMA-transpose xbar (off-engine)
    psvt = st.psA.tile([128, 512], F32, name="psvt", tag="psA")
    for kc in range(KCH):
        _mm2(nc, psvt, st.wv_s[:, kc, :], xt[:, kc, :],
             (kc == 0), (kc == KCH - 1))
    vt = st.vtpool.tile([128, 512], F16, name="vt")
    nc.vector.tensor_copy(vt[:], psvt[:])
    # bounce through DRAM: the transpose xbar only supports writes from a
    # DRAM source reliably (SBUF->SBUF transpose wedges the device)
    nc.sync.dma_start(st.vtd[n], vt[:])
    for s in range(4):
        nc.sync.dma_start(st.vv[:, n * 4 + s, :],
                          st.vtd[n][:, s * 128:(s + 1) * 128], transpose=True)


def _emit_p2_qc(nc, st, b, h, qc):
    """Attention for one (batch, head, 512-wide q chunk)."""
    mult = mybir.AluOpType.mult
    base = b * T
    q0 = base + qc * 512
    nkc = 4 * (qc + 1)  # causal: k chunks 0..nkc-1
    nquad = nkc // 4
    psv2 = st.psB.tile([128, 512], F32, name="psv2", tag="psB")
    psd = st.psB.tile([128, 512], F32, name="psd", tag="psB")
    pts = [None] * nquad

    def emit_scores(qi):
        diag = qi == nquad - 1
        pss = st.psS.tile([128, 4, 512], F32, name="pss")
        pt = st.ppool.tile([128, 4, 512], F16, name="pt")
        for i in range(4):
            kc = qi * 4 + i
            off = i * 128 if diag else 0  # causal: q >= kc*128 only
            _mm2(nc, pss[:, i, off:512],
                 st.kT[:, base + kc * 128:base + (kc + 1) * 128],
                 st.qT[:, h, q0 + off:q0 + 512], True, True)
        nc.scalar.activation(pt[:], pss[:], mybir.ActivationFunctionType.Exp)
        if diag:  # mask the diagonal tiles (same pattern at every offset)
            for i in range(4):
                off = i * 128
                nc.vector.tensor_tensor(
                    pt[:, i, off:512], pt[:, i, off:512],
                    st.masks_s[:, 0, 0:512 - off], mult)
        pts[qi] = pt

    def emit_consume(qi):
        diag = qi == nquad - 1
        pt = pts[qi]
        for i in range(4):
            kc = qi * 4 + i
            off = i * 128 if diag else 0
            first = (kc == 0)
            last = (kc == nkc - 1)
            nc.tensor.matmul(
                psd[0:1, off:512], lhsT=st.ones_s[:],
                rhs=pt[:, i, off:512], start=first, stop=last)
            _mm2(nc, psv2[:, off:512], st.vv[:, b * KC + kc, :],
                 pt[:, i, off:512], first, last)

    # 1-stage software pipeline: PE's consume matmuls never sit
    # immediately behind the exp they wait on
    for qi in range(nquad + 1):
        if qi < nquad:
            emit_scores(qi)
        if qi > 0:
            emit_consume(qi - 1)

    # evict both accumulators immediately so the psB ring never blocks the
    # next q-chunk; normalization happens out of PSUM
    oraw = st.opool.tile([128, 512], F16, name="oraw")
    nc.vector.tensor_copy(oraw[:], psv2[:])
    recip = st.rpool.tile([1, 512], F32, name="recip")
    nc.vector.reciprocal(recip[:], psd[0:1, :])
    bc = st.bpool.tile([128, 512], F32, name="bc")
    nc.gpsimd.partition_broadcast(bc[:], recip[:])
    nc.vector.tensor_tensor(st.oT[:, h, q0:q0 + 512], oraw[:], bc[:], mult)


def _emit_p3_group(nc, st, b, qc):
    """Output projection for the 4 token tiles covered by (b, qc)."""
    for i in range(4):
        t0 = b * T + qc * 512 + i * 128
        for cc in range(C // 512):
            c0 = cc * 512
            pso = st.psA.tile([128, 512], F32, name="pso", tag="psA")
            for h in range(HL):
                _mm2(nc, pso, st.oT[:, h, t0:t0 + 128],
                     st.wo_s[:, h, c0:c0 + 512], (h == 0), (h == HL - 1))
            yt = st.ypool.tile([128, 512], F16, name="yt")
            # balance PSUM eviction between ACT and DVE
            if st.p3_idx % 2 == 0:
                nc.scalar.copy(yt[:], pso[:])
            else:
                nc.vector.tensor_copy(yt[:], pso[:])
            dma_eng = nc.sync if st.p3_idx % 2 == 0 else nc.scalar
            st.p3_idx += 1
            dma_eng.dma_start(st.y[t0:t0 + 128, c0:c0 + 512], yt[:])


def _emit_cc(nc, st):
    """Pair K/V exchange: ship own half, receive partner half."""
    w1 = nc.sync.dma_start(st.kv_loc[:, 0:T], st.kT[:, 0:T])
    w2 = nc.sync.dma_start(
        st.kv_loc[:, T:2 * T],
        st.vv[:, 0:KC, :].rearrange("p a b -> p (a b)"))
    cc = nc.gpsimd.collective_compute(
        "AllGather",
        mybir.AluOpType.bypass,
        replica_groups=st.replica_groups,
        ins=[st.kv_loc[:]],
        outs=[st.kv_all[:]],
    )
    add_dep_helper(cc.ins, w1.ins, reason="cc after kv_loc k write")
    add_dep_helper(cc.ins, w2.ins, reason="cc after kv_loc v write")
    # partner slot in the gathered buffer is rank-relative: 1 - (pid & 1)
    partner = 1 - (nc.partition_id() & 1)
    prow = partner * 128
    r1 = nc.sync.dma_start(st.kT[:, T:2 * T],
                           st.kv_all[bass.ds(prow, 128), 0:T])
    r2 = nc.sync.dma_start(
        st.vv[:, KC:2 * KC, :].rearrange("p a b -> p (a b)"),
        st.kv_all[bass.ds(prow, 128), T:2 * T])
    add_dep_helper(r1.ins, cc.ins, reason="k read after cc")
    add_dep_helper(r2.ins, cc.ins, reason="v read after cc")


def _emit_body(nc, st, emit_cc=True):
    """One full forward pass, phases interleaved so ACT's exp work overlaps
    PE-heavy projection / output-projection stretches.

    Batch slots are core-local: slot 0 is this core's own batch (K/V
    computed locally), slot 1 is the partner core's batch (K/V arrive via
    the pair AllGather). The host un-permutes odd cores' outputs."""
    st.p3_idx = 0
    for n in range(QC):  # own-batch token chunks: full q/k/v
        _emit_p1_chunk(nc, st, n, with_kv=True)
    if st.use_cc and emit_cc:
        _emit_cc(nc, st)
    for qc in range(QC):  # slot-0/h0 attention overlaps slot-1 projections
        _emit_p2_qc(nc, st, 0, 0, qc)
        _emit_p1_chunk(nc, st, QC + qc, with_kv=not st.use_cc)
    for qc in range(QC):  # slot-0/h1 attention overlaps slot-0 out-proj
        _emit_p2_qc(nc, st, 0, 1, qc)
        _emit_p3_group(nc, st, 0, qc)
    for qc in range(QC):  # slot-1 attention (both heads) + slot-1 out-proj
        _emit_p2_qc(nc, st, 1, 0, qc)
        _emit_p2_qc(nc, st, 1, 1, qc)
        _emit_p3_group(nc, st, 1, qc)
    if st.debug_kv:
        nc.sync.dma_start(st.kt_dbg[:], st.kT[:])
        nc.sync.dma_start(st.vv_dbg[:],
                          st.vv[:].rearrange("p a b -> p (a b)"))


def _build_program(reps=1, loop_n=None):
    nc = bacc.Bacc("TRN2", target_bir_lowering=False, debug=False,
                   num_devices=N_CORES)

    # all inputs arrive pre-packed so every DMA is a contiguous burst
    xTp = nc.dram_tensor("xTp", [NCH, 128, KCH, 512], F16,
                         kind="ExternalInput").ap()
    wq = nc.dram_tensor("wq", [128, KCH, HL * D], F16,
                        kind="ExternalInput").ap()
    wk = nc.dram_tensor("wk", [128, KCH, D], F16, kind="ExternalInput").ap()
    wv = nc.dram_tensor("wv", [128, KCH, D], F16, kind="ExternalInput").ap()
    wo = nc.dram_tensor("wo", [128, HL, C], F16, kind="ExternalInput").ap()
    cosq = nc.dram_tensor("cosq", [D, T], F16, kind="ExternalInput").ap()
    sinq = nc.dram_tensor("sinq", [D, T], F16, kind="ExternalInput").ap()
    cosk = nc.dram_tensor("cosk", [D, T], F16, kind="ExternalInput").ap()
    sink = nc.dram_tensor("sink", [D, T], F16, kind="ExternalInput").ap()
    masks = nc.dram_tensor("masks", [128, 4, 512], F16, kind="ExternalInput").ap()
    y = nc.dram_tensor("y", [BT, C], F16, kind="ExternalOutput").ap()
    vtd = nc.dram_tensor("vtd", [NCH, D, 512], F16).ap()
    kv_loc = nc.dram_tensor("kv_loc", [D, 2 * T], F16).ap()
    kv_all = nc.dram_tensor("kv_all", [2 * D, 2 * T], F16).ap()
    if DEBUG_KV:
        kt_dbg = nc.dram_tensor("kt_dbg", [D, BT], F16,
                                kind="ExternalOutput").ap()
        vv_dbg = nc.dram_tensor("vv_dbg", [128, BT], F16,
                                kind="ExternalOutput").ap()

    st = _St()
    with tile.TileContext(nc) as tc:
        with (
            tc.tile_pool(name="cpool", bufs=1) as cpool,
            tc.tile_pool(name="xpool", bufs=2) as xpool,
            tc.tile_pool(name="ppool", bufs=4) as ppool,
            tc.tile_pool(name="rpool", bufs=3) as rpool,
            tc.tile_pool(name="bpool", bufs=2) as bpool,
            tc.tile_pool(name="opool", bufs=2) as opool,
            tc.tile_pool(name="vtpool", bufs=2) as vtpool,
            tc.tile_pool(name="ypool", bufs=8) as ypool,
            tc.tile_pool(name="psA", bufs=2, space="PSUM") as psA,
            tc.tile_pool(name="psB", bufs=2, space="PSUM") as psB,
            tc.tile_pool(name="psS", bufs=1, space="PSUM") as psS,
        ):
            st.xpool, st.ppool, st.rpool, st.bpool, st.ypool = (
                xpool, ppool, rpool, bpool, ypool)
            st.opool = opool
            st.vtpool = vtpool
            st.psA, st.psB, st.psS = psA, psB, psS

            # ---- persistent SBUF state ----
            st.wq_s = cpool.tile([128, KCH, HL * D], F16, name="wq_s")
            st.wk_s = cpool.tile([128, KCH, D], F16, name="wk_s")
            st.wv_s = cpool.tile([128, KCH, D], F16, name="wv_s")
            st.wo_s = cpool.tile([128, HL, C], F16, name="wo_s")
            st.cosq_s = cpool.tile([D, T], F16, name="cosq_s")
            st.sinq_s = cpool.tile([D, T], F16, name="sinq_s")
            st.cosk_s = cpool.tile([D, T], F16, name="cosk_s")
            st.sink_s = cpool.tile([D, T], F16, name="sink_s")
            st.masks_s = cpool.tile([128, 4, 512], F16, name="masks_s")
            st.ones_s = cpool.tile([128, 1], F16, name="ones_s")
            st.qT = cpool.tile([D, HL, BT], F16, name="qT")
            st.kT = cpool.tile([D, BT], F16, name="kT")
            st.vv = cpool.tile([128, BT // 128, D], F16, name="vv")
            st.oT = cpool.tile([D, HL, BT], F16, name="oT")

            # ordered by first-use time so the first matmul starts ASAP:
            # wq+xt0 gate matmul #1, rope tables gate the first evictions,
            # masks/wo are not needed until attention / output projection
            nc.sync.dma_start(st.wq_s[:], wq[:])
            nc.scalar.dma_start(st.cosq_s[:], cosq[:])
            nc.scalar.dma_start(st.sinq_s[:], sinq[:])
            nc.scalar.dma_start(st.wk_s[:], wk[:])
            nc.scalar.dma_start(st.cosk_s[:], cosk[:])
            nc.scalar.dma_start(st.sink_s[:], sink[:])
            nc.scalar.dma_start(st.wv_s[:], wv[:])
            nc.scalar.dma_start(st.masks_s[:], masks[:])
            nc.scalar.dma_start(st.wo_s[:], wo[:])
            nc.vector.memset(st.ones_s[:], 1.0)

            st.xTp = xTp
            st.y = y
            st.vtd = vtd
            st.kv_loc = kv_loc
            st.kv_all = kv_all
            st.use_cc = USE_CC
            st.replica_groups = [[2 * i, 2 * i + 1]
                                 for i in range(N_CORES // 2)]
            st.debug_kv = DEBUG_KV
            if DEBUG_KV:
                st.kt_dbg, st.vv_dbg = kt_dbg, vv_dbg

            if loop_n is not None:
                engs = (mybir.EngineType.PE, mybir.EngineType.Activation,
                        mybir.EngineType.DVE, mybir.EngineType.SP,
                        mybir.EngineType.Pool)
                if USE_CC:
                    # collectives desync inside For_i; run the exchange once
                    # before the loop (its cost hides behind q projections)
                    _emit_cc(nc, st)
                with tc.For_i(0, loop_n, 1, hint_engines=engs):
                    _emit_body(nc, st, emit_cc=False)
            else:
                for _rep in range(reps):
                    _emit_body(nc, st)

    nc.compile()
    return nc


def _rope_tables():
    inv = (1.0 / (10000.0 ** (np.arange(0, D, 2, dtype=np.float32) / D)))
    f = np.arange(T, dtype=np.float32)[:, None] * inv[None, :]  # [T, 64]
    cos = np.concatenate([np.cos(f)] * 2, axis=-1).astype(np.float32)  # [T,128]
    sin = np.concatenate([np.sin(f)] * 2, axis=-1).astype(np.float32)
    sgn = np.ones((D,), np.float32)
    sgn[0:64] = -1.0  # sign-folded for the rotate-half formulation
    sinf = sin * sgn[None, :]
    alpha = np.float32(1.0 / np.sqrt(D))
    return (cos.T.copy() * alpha, sinf.T.copy() * alpha,
            cos.T.copy(), sinf.T.copy())


def _masks():
    i = np.arange(128)[:, None]
    j = np.arange(512)[None, :]
    m = np.stack([(j >= i + 128 * d) for d in range(4)], axis=0)  # [4,128,512]
    return np.ascontiguousarray(m.transpose(1, 0, 2)).astype(np.float16)


def _pack_w(w):
    """[K*128, M] -> [128, K, M] (per-partition contiguous)."""
    kch, m = w.shape[0] // 128, w.shape[1]
    return np.ascontiguousarray(
        w.reshape(kch, 128, m).transpose(1, 0, 2)).astype(np.float16)


def make_in_maps(x, Wq, Wk, Wv, Wo):
    # xTp[n, p, k, j] = x^T[k*128+p, n*512+j]
    xT = x.reshape(BT, C).T.astype(np.float16)  # [C, BT]
    xTp = np.ascontiguousarray(
        xT.reshape(KCH, 128, NCH, 512).transpose(2, 1, 0, 3))
    # under the CC scheme, odd cores see their own batch (b1) in slot 0
    xTp_odd = (np.ascontiguousarray(np.roll(xTp, QC, axis=0))
               if USE_CC else xTp)
    cq, sq, ck, sk = [t.astype(np.float16) for t in _rope_tables()]
    mk = _masks()
    in_maps = []
    for c in range(N_CORES):
        g = c // 2  # kv head for this core's 2 q-heads
        in_maps.append({
            "xTp": xTp if c % 2 == 0 else xTp_odd,
            "wq": _pack_w(Wq[:, c * HL * D:(c + 1) * HL * D]),
            "wk": _pack_w(Wk[:, g * D:(g + 1) * D]),
            "wv": _pack_w(Wv[:, g * D:(g + 1) * D]),
            "wo": _pack_w(Wo[c * HL * D:(c + 1) * HL * D, :]),
            "cosq": cq, "sinq": sq, "cosk": ck, "sink": sk,
            "masks": mk,
        })
    return in_maps


def get_program(reps=1, loop_n=None):
    key = ("nc", reps, loop_n)
    if key not in _cache:
        _cache[key] = _build_program(reps, loop_n)
    return _cache[key]


def kernel(x, Wq, Wk, Wv, Wo):
    nc = get_program()
    in_maps = make_in_maps(x, Wq, Wk, Wv, Wo)
    res = run_bass_kernel_spmd(nc, in_maps, core_ids=list(range(N_CORES)))
    out = np.zeros((BT, C), np.float32)
    for c in range(N_CORES):
        yc = res.results[c]["y"].astype(np.float32)
        if USE_CC and c % 2 == 1:  # un-permute batch slots
            yc = np.concatenate([yc[T:], yc[:T]], axis=0)
        out += yc
    return out.reshape(B, T, C)


if __name__ == "__main__":
    rng = np.random.default_rng(0)
    x = rng.standard_normal((B, T, C), dtype=np.float32)
    Wq = rng.standard_normal((C, H * D), dtype=np.float32) * 0.02
    Wk = rng.standard_normal((C, KV * D), dtype=np.float32) * 0.02
    Wv = rng.standard_normal((C, KV * D), dtype=np.float32) * 0.02
    Wo = rng.standard_normal((C, C), dtype=np.float32) * 0.02
    out = kernel(x=x, Wq=Wq, Wk=Wk, Wv=Wv, Wo=Wo)
    print("out", out.shape, out.dtype, float(np.abs(out).max()))


# revision 62
# speedup vs baseline: 1.0026x; 1.0026x over previous
"""GQA attention (B=2,T=2048,C=2048,H=16,KV=4,D=128) + RoPE + causal softmax
+ output projection, tensor-parallel over 8 NeuronCores (2 q-heads/core).

Contract: kernel(**inputs) takes full numpy inputs, returns full output.
Per-core partial outputs (o @ Wo[rows]) are summed on the host.
"""

import sys

sys.path.insert(0, "/opt/trn_rl_repo")

import numpy as np

import concourse.bacc as bacc
import concourse.bass as bass
import concourse.mybir as mybir
import concourse.tile as tile
from concourse.tile import add_dep_helper
from concourse.bass_utils import run_bass_kernel_spmd

B, T, C = 2, 2048, 2048
H, KV, D = 16, 4, 128
G = H // KV
N_CORES = 8
HL = H // N_CORES  # 2 q-heads per core
BT = B * T  # 4096
NCH = BT // 512  # 8 token chunks of 512
KCH = C // 128  # 16 contraction chunks
QC = T // 512  # 4 q chunks per batch
KC = T // 128  # 16 k chunks per batch

F16 = mybir.dt.float16
F32 = mybir.dt.float32

_cache = {}


class _St:
    pass


SPLIT_MM = False
USE_CC = True
DEBUG_KV = False


def _mm2(nc, out, lhsT, rhs, start, stop):
    """M=128 matmul, optionally as two col-tiled M=64 halves (the second
    LDWEIGHTS can overlap the first matmul in the other column group)."""
    if not SPLIT_MM:
        nc.tensor.matmul(out[:, :], lhsT=lhsT, rhs=rhs,
                         start=start, stop=stop)
        return
    nc.tensor.matmul(out[0:64, :], lhsT=lhsT[:, 0:64], rhs=rhs,
                     start=start, stop=stop)
    nc.tensor.matmul(out[64:128, :], lhsT=lhsT[:, 64:128], rhs=rhs,
                     start=start, stop=stop, tile_position=(0, 64))


def _emit_p1_chunk(nc, st, n, with_kv=True):
    """QKV projections for one 512-token chunk.

    K/V are only computed for the core's own batch half (chunks 0..3);
    the partner core's half arrives via a 2-core AllGather."""
    mult = mybir.AluOpType.mult

    def rope(dst, ps, cos_s, sin_s, t0):
        t1 = st.rpool.tile([128, 512], F32, name="t1")
        t2 = st.rpool.tile([128, 512], F32, name="t2")
        nc.vector.tensor_tensor(t1[:], ps[:], cos_s[:, t0:t0 + 512], mult)
        nc.vector.tensor_tensor(t2[0:64, :], ps[64:128, :],
                                sin_s[0:64, t0:t0 + 512], mult)
        nc.vector.tensor_tensor(t2[64:128, :], ps[0:64, :],
                                sin_s[64:128, t0:t0 + 512], mult)
        nc.vector.tensor_tensor(dst, t1[:], t2[:], mybir.AluOpType.add)

    n0 = n * 512
    t0 = (n % QC) * 512  # rope-table offset (within batch)
    xt = st.xpool.tile([128, KCH, 512], F16, name="xt")
    dma_eng = nc.sync if n % 2 == 0 else nc.scalar
    dma_eng.dma_start(xt[:], st.xTp[n])
    for h in range(HL):
        psq = st.psA.tile([128, 512], F32, name="psq", tag="psA")
        for kc in range(KCH):
            _mm2(nc, psq, st.wq_s[:, kc, h * D:(h + 1) * D],
                 xt[:, kc, :], (kc == 0), (kc == KCH - 1))
        rope(st.qT[:, h, n0:n0 + 512], psq, st.cosq_s, st.sinq_s, t0)
    if not with_kv:
        return
    psk = st.psA.tile([128, 512], F32, name="psk", tag="psA")
    for kc in range(KCH):
        _mm2(nc, psk, st.wk_s[:, kc, :], xt[:, kc, :],
             (kc == 0), (kc == KCH - 1))
    rope(st.kT[:, n0:n0 + 512], psk, st.cosk_s, st.sink_s, t0)
    # v computed in transposed layout (N=512 matmuls), then moved to the
    # natural [token, d] layout via the DMA-transpose xbar (off-engine)
    psvt = st.psA.tile([128, 512], F32, name="psvt", tag="psA")
    for kc in range(KCH):
        _mm2(nc, psvt, st.wv_s[:, kc, :], xt[:, kc, :],
             (kc == 0), (kc == KCH - 1))
    vt = st.vtpool.tile([128, 512], F16, name="vt")
    nc.vector.tensor_copy(vt[:], psvt[:])
    # bounce through DRAM: the transpose xbar only supports writes from a
    # DRAM source reliably (SBUF->SBUF transpose wedges the device)
    nc.sync.dma_start(st.vtd[n], vt[:])
    for s in range(4):
        nc.sync.dma_start(st.vv[:, n * 4 + s, :],
                          st.vtd[n][:, s * 128:(s + 1) * 128], transpose=True)


def _emit_p2_qc(nc, st, b, h, qc):
    """Attention for one (batch, head, 512-wide q chunk)."""
    mult = mybir.AluOpType.mult
    base = b * T
    q0 = base + qc * 512
    nkc = 4 * (qc + 1)  # causal: k chunks 0..nkc-1
    nquad = nkc // 4
    psv2 = st.psB.tile([128, 512], F32, name="psv2", tag="psB")
    psd = st.psB.tile([128, 512], F32, name="psd", tag="psB")
    pts = [None] * nquad

    def emit_scores(qi):
        diag = qi == nquad - 1
        pss = st.psS.tile([128, 4, 512], F32, name="pss")
        pt = st.ppool.tile([128, 4, 512], F16, name="pt")
        for i in range(4):
            kc = qi * 4 + i
            off = i * 128 if diag else 0  # causal: q >= kc*128 only
            _mm2(nc, pss[:, i, off:512],
                 st.kT[:, base + kc * 128:base + (kc + 1) * 128],
                 st.qT[:, h, q0 + off:q0 + 512], True, True)
        nc.scalar.activation(pt[:], pss[:], mybir.ActivationFunctionType.Exp)
        if diag:  # mask the diagonal tiles (same pattern at every offset)
            for i in range(4):
                off = i * 128
                nc.vector.tensor_tensor(
                    pt[:, i, off:512], pt[:, i, off:512],
                    st.masks_s[:, 0, 0:512 - off], mult)
        pts[qi] = pt

    def emit_consume(qi):
        diag = qi == nquad - 1
        pt = pts[qi]
        for i in range(4):
            kc = qi * 4 + i
            off = i * 128 if diag else 0
            first = (kc == 0)
            last = (kc == nkc - 1)
            nc.tensor.matmul(
                psd[0:1, off:512], lhsT=st.ones_s[:],
                rhs=pt[:, i, off:512], start=first, stop=last)
            _mm2(nc, psv2[:, off:512], st.vv[:, b * KC + kc, :],
                 pt[:, i, off:512], first, last)

    # 1-stage software pipeline: PE's consume matmuls never sit
    # immediately behind the exp they wait on
    for qi in range(nquad + 1):
        if qi < nquad:
            emit_scores(qi)
        if qi > 0:
            emit_consume(qi - 1)

    # evict both accumulators immediately so the psB ring never blocks the
    # next q-chunk; normalization happens out of PSUM
    oraw = st.opool.tile([128, 512], F16, name="oraw")
    nc.vector.tensor_copy(oraw[:], psv2[:])
    recip = st.rpool.tile([1, 512], F32, name="recip")
    nc.vector.reciprocal(recip[:], psd[0:1, :])
    bc = st.bpool.tile([128, 512], F32, name="bc")
    nc.gpsimd.partition_broadcast(bc[:], recip[:])
    nc.vector.tensor_tensor(st.oT[:, h, q0:q0 + 512], oraw[:], bc[:], mult)


def _emit_p3_group(nc, st, b, qc):
    """Output projection for the 4 token tiles covered by (b, qc)."""
    for i in range(4):
        t0 = b * T + qc * 512 + i * 128
        for cc in range(C // 512):
            c0 = cc * 512
            pso = st.psA.tile([128, 512], F32, name="pso", tag="psA")
            for h in range(HL):
                _mm2(nc, pso, st.oT[:, h, t0:t0 + 128],
                     st.wo_s[:, h, c0:c0 + 512], (h == 0), (h == HL - 1))
            yt = st.ypool.tile([128, 512], F16, name="yt")
            # balance PSUM eviction between ACT and DVE
            if st.p3_idx % 2 == 0:
                nc.scalar.copy(yt[:], pso[:])
            else:
                nc.vector.tensor_copy(yt[:], pso[:])
            dma_eng = nc.sync if st.p3_idx % 2 == 0 else nc.scalar
            st.p3_idx += 1
            dma_eng.dma_start(st.y[t0:t0 + 128, c0:c0 + 512], yt[:])


def _emit_cc(nc, st):
    """Pair K/V exchange: ship own half, receive partner half."""
    w1 = nc.sync.dma_start(st.kv_loc[:, 0:T], st.kT[:, 0:T])
    w2 = nc.sync.dma_start(
        st.kv_loc[:, T:2 * T],
        st.vv[:, 0:KC, :].rearrange("p a b -> p (a b)"))
    cc = nc.gpsimd.collective_compute(
        "AllGather",
        mybir.AluOpType.bypass,
        replica_groups=st.replica_groups,
        ins=[st.kv_loc[:]],
        outs=[st.kv_all[:]],
    )
    add_dep_helper(cc.ins, w1.ins, reason="cc after kv_loc k write")
    add_dep_helper(cc.ins, w2.ins, reason="cc after kv_loc v write")
    # partner slot in the gathered buffer is rank-relative: 1 - (pid & 1)
    partner = 1 - (nc.partition_id() & 1)
    prow = partner * 128
    r1 = nc.sync.dma_start(st.kT[:, T:2 * T],
                           st.kv_all[bass.ds(prow, 128), 0:T])
    r2 = nc.sync.dma_start(
        st.vv[:, KC:2 * KC, :].rearrange("p a b -> p (a b)"),
        st.kv_all[bass.ds(prow, 128), T:2 * T])
    add_dep_helper(r1.ins, cc.ins, reason="k read after cc")
    add_dep_helper(r2.ins, cc.ins, reason="v read after cc")


def _emit_body(nc, st, emit_cc=True):
    """One full forward pass, phases interleaved so ACT's exp work overlaps
    PE-heavy projection / output-projection stretches.

    Batch slots are core-local: slot 0 is this core's own batch (K/V
    computed locally), slot 1 is the partner core's batch (K/V arrive via
    the pair AllGather). The host un-permutes odd cores' outputs."""
    st.p3_idx = 0
    for n in range(QC):  # own-batch token chunks: full q/k/v
        _emit_p1_chunk(nc, st, n, with_kv=True)
    if st.use_cc and emit_cc:
        _emit_cc(nc, st)
    for qc in range(QC):  # slot-0/h0 attention overlaps slot-1 projections
        _emit_p2_qc(nc, st, 0, 0, qc)
        _emit_p1_chunk(nc, st, QC + qc, with_kv=not st.use_cc)
    for qc in range(QC):  # slot-0/h1 attention overlaps slot-0 out-proj
        _emit_p2_qc(nc, st, 0, 1, qc)
        _emit_p3_group(nc, st, 0, qc)
    for qc in range(QC):  # slot-1 attention (both heads) + slot-1 out-proj
        _emit_p2_qc(nc, st, 1, 0, qc)
        _emit_p2_qc(nc, st, 1, 1, qc)
        _emit_p3_group(nc, st, 1, qc)
    if st.debug_kv:
        nc.sync.dma_start(st.kt_dbg[:], st.kT[:])
        nc.sync.dma_start(st.vv_dbg[:],
                          st.vv[:].rearrange("p a b -> p (a b)"))


def _build_program(reps=1, loop_n=None):
    nc = bacc.Bacc("TRN2", target_bir_lowering=False, debug=False,
                   num_devices=N_CORES)

    # all inputs arrive pre-packed so every DMA is a contiguous burst
    xTp = nc.dram_tensor("xTp", [NCH, 128, KCH, 512], F16,
                         kind="ExternalInput").ap()
    wq = nc.dram_tensor("wq", [128, KCH, HL * D], F16,
                        kind="ExternalInput").ap()
    wk = nc.dram_tensor("wk", [128, KCH, D], F16, kind="ExternalInput").ap()
    wv = nc.dram_tensor("wv", [128, KCH, D], F16, kind="ExternalInput").ap()
    wo = nc.dram_tensor("wo", [128, HL, C], F16, kind="ExternalInput").ap()
    cosq = nc.dram_tensor("cosq", [D, T], F16, kind="ExternalInput").ap()
    sinq = nc.dram_tensor("sinq", [D, T], F16, kind="ExternalInput").ap()
    cosk = nc.dram_tensor("cosk", [D, T], F16, kind="ExternalInput").ap()
    sink = nc.dram_tensor("sink", [D, T], F16, kind="ExternalInput").ap()
    masks = nc.dram_tensor("masks", [128, 4, 512], F16, kind="ExternalInput").ap()
    y = nc.dram_tensor("y", [BT, C], F16, kind="ExternalOutput").ap()
    vtd = nc.dram_tensor("vtd", [NCH, D, 512], F16).ap()
    kv_loc = nc.dram_tensor("kv_loc", [D, 2 * T], F16).ap()
    kv_all = nc.dram_tensor("kv_all", [2 * D, 2 * T], F16).ap()
    if DEBUG_KV:
        kt_dbg = nc.dram_tensor("kt_dbg", [D, BT], F16,
                                kind="ExternalOutput").ap()
        vv_dbg = nc.dram_tensor("vv_dbg", [128, BT], F16,
                                kind="ExternalOutput").ap()

    st = _St()
    with tile.TileContext(nc) as tc:
        with (
            tc.tile_pool(name="cpool", bufs=1) as cpool,
            tc.tile_pool(name="xpool", bufs=2) as xpool,
            tc.tile_pool(name="ppool", bufs=4) as ppool,
            tc.tile_pool(name="rpool", bufs=3) as rpool,
            tc.tile_pool(name="bpool", bufs=2) as bpool,
            tc.tile_pool(name="opool", bufs=2) as opool,
            tc.tile_pool(name="vtpool", bufs=2) as vtpool,
            tc.tile_pool(name="ypool", bufs=8) as ypool,
            tc.tile_pool(name="psA", bufs=2, space="PSUM") as psA,
            tc.tile_pool(name="psB", bufs=2, space="PSUM") as psB,
            tc.tile_pool(name="psS", bufs=1, space="PSUM") as psS,
        ):
            st.xpool, st.ppool, st.rpool, st.bpool, st.ypool = (
                xpool, ppool, rpool, bpool, ypool)
            st.opool = opool
            st.vtpool = vtpool
            st.psA, st.psB, st.psS = psA, psB, psS

            # ---- persistent SBUF state ----
            st.wq_s = cpool.tile([128, KCH, HL * D], F16, name="wq_s")
            st.wk_s = cpool.tile([128, KCH, D], F16, name="wk_s")
            st.wv_s = cpool.tile([128, KCH, D], F16, name="wv_s")
            st.wo_s = cpool.tile([128, HL, C], F16, name="wo_s")
            st.cosq_s = cpool.tile([D, T], F16, name="cosq_s")
            st.sinq_s = cpool.tile([D, T], F16, name="sinq_s")
            st.cosk_s = cpool.tile([D, T], F16, name="cosk_s")
            st.sink_s = cpool.tile([D, T], F16, name="sink_s")
            st.masks_s = cpool.tile([128, 4, 512], F16, name="masks_s")
            st.ones_s = cpool.tile([128, 1], F16, name="ones_s")
            st.qT = cpool.tile([D, HL, BT], F16, name="qT")
            st.kT = cpool.tile([D, BT], F16, name="kT")
            st.vv = cpool.tile([128, BT // 128, D], F16, name="vv")
            st.oT = cpool.tile([D, HL, BT], F16, name="oT")

            # ordered by first-use time so the first matmul starts ASAP:
            # wq+xt0 gate matmul #1, rope tables gate the first evictions,
            # masks/wo are not needed until attention / output projection
            nc.sync.dma_start(st.wq_s[:], wq[:])
            nc.scalar.dma_start(st.cosq_s[:], cosq[:])
            nc.scalar.dma_start(st.sinq_s[:], sinq[:])
            nc.scalar.dma_start(st.wk_s[:], wk[:])
            nc.scalar.dma_start(st.cosk_s[:], cosk[:])
            nc.scalar.dma_start(st.sink_s[:], sink[:])
            nc.scalar.dma_start(st.wv_s[:], wv[:])
            nc.scalar.dma_start(st.masks_s[:], masks[:])
            nc.scalar.dma_start(st.wo_s[:], wo[:])
            nc.vector.memset(st.ones_s[:], 1.0)

            st.xTp = xTp
            st.y = y
            st.vtd = vtd
            st.kv_loc = kv_loc
            st.kv_all = kv_all
            st.use_cc = USE_CC
            st.replica_groups = [[2 * i, 2 * i + 1]
                                 for i in range(N_CORES // 2)]
            st.debug_kv = DEBUG_KV
            if DEBUG_KV:
                st.kt_dbg, st.vv_dbg = kt_dbg, vv_dbg

            if loop_n is not None:
                engs = (mybir.EngineType.PE, mybir.EngineType.Activation,
                        mybir.EngineType.DVE, mybir.EngineType.SP,
                        mybir.EngineType.Pool)
                if USE_CC:
                    # collectives desync inside For_i; run the exchange once
                    # before the loop (its cost hides behind q projections)
                    _emit_cc(nc, st)
                with tc.For_i(0, loop_n, 1, hint_engines=engs):
                    _emit_body(nc, st, emit_cc=False)
            else:
                for _rep in range(reps):
                    _emit_body(nc, st)

    nc.compile()
    return nc


def _rope_tables():
    inv = (1.0 / (10000.0 ** (np.arange(0, D, 2, dtype=np.float32) / D)))
    f = np.arange(T, dtype=np.float32)[:, None] * inv[None, :]  # [T, 64]
    cos = np.concatenate([np.cos(f)] * 2, axis=-1).astype(np.float32)  # [T,128]
    sin = np.concatenate([np.sin(f)] * 2, axis=-1).astype(np.float32)
    sgn = np.ones((D,), np.float32)
    sgn[0:64] = -1.0  # sign-folded for the rotate-half formulation
    sinf = sin * sgn[None, :]
    alpha = np.float32(1.0 / np.sqrt(D))
    return (cos.T.copy() * alpha, sinf.T.copy() * alpha,
            cos.T.copy(), sinf.T.copy())


def _masks():
    i = np.arange(128)[:, None]
    j = np.arange(512)[None, :]
    m = np.stack([(j >= i + 128 * d) for d in range(4)], axis=0)  # [4,128,512]
    return np.ascontiguousarray(m.transpose(1, 0, 2)).astype(np.float16)


def _pack_w(w):
    """[K*128, M] -> [128, K, M] (per-partition contiguous)."""
    kch, m = w.shape[0] // 128, w.shape[1]
    return np.ascontiguousarray(
        w.reshape(kch, 128, m).transpose(1, 0, 2)).astype(np.float16)


def make_in_maps(x, Wq, Wk, Wv, Wo):
    # xTp[n, p, k, j] = x^T[k*128+p, n*512+j]
    xT = x.reshape(BT, C).T.astype(np.float16)  # [C, BT]
    xTp = np.ascontiguousarray(
        xT.reshape(KCH, 128, NCH, 512).transpose(2, 1, 0, 3))
    # under the CC scheme, odd cores see their own batch (b1) in slot 0
    xTp_odd = (np.ascontiguousarray(np.roll(xTp, QC, axis=0))
               if USE_CC else xTp)
    cq, sq, ck, sk = [t.astype(np.float16) for t in _rope_tables()]
    mk = _masks()
    in_maps = []
    for c in range(N_CORES):
        g = c // 2  # kv head for this core's 2 q-heads
        in_maps.append({
            "xTp": xTp if c % 2 == 0 else xTp_odd,
            "wq": _pack_w(Wq[:, c * HL * D:(c + 1) * HL * D]),
            "wk": _pack_w(Wk[:, g * D:(g + 1) * D]),
            "wv": _pack_w(Wv[:, g * D:(g + 1) * D]),
            "wo": _pack_w(Wo[c * HL * D:(c + 1) * HL * D, :]),
            "cosq": cq, "sinq": sq, "cosk": ck, "sink": sk,
            "masks": mk,
        })
    return in_maps


def get_program(reps=1, loop_n=None):
    key = ("nc", reps, loop_n)
    if key not in _cache:
        _cache[key] = _build_program(reps, loop_n)
    return _cache[key]


def kernel(x, Wq, Wk, Wv, Wo):
    # coerce to host numpy up front (harness may pass jax arrays)
    x, Wq, Wk, Wv, Wo = (np.asarray(a, dtype=np.float32)
                         for a in (x, Wq, Wk, Wv, Wo))
    nc = get_program()
    in_maps = make_in_maps(x, Wq, Wk, Wv, Wo)
    res = run_bass_kernel_spmd(nc, in_maps, core_ids=list(range(N_CORES)))
    out = np.zeros((BT, C), np.float32)
    for c in range(N_CORES):
        yc = res.results[c]["y"].astype(np.float32)
        if USE_CC and c % 2 == 1:  # un-permute batch slots
            yc = np.concatenate([yc[T:], yc[:T]], axis=0)
        out += yc
    return out.reshape(B, T, C)


if __name__ == "__main__":
    rng = np.random.default_rng(0)
    x = rng.standard_normal((B, T, C), dtype=np.float32)
    Wq = rng.standard_normal((C, H * D), dtype=np.float32) * 0.02
    Wk = rng.standard_normal((C, KV * D), dtype=np.float32) * 0.02
    Wv = rng.standard_normal((C, KV * D), dtype=np.float32) * 0.02
    Wo = rng.standard_normal((C, C), dtype=np.float32) * 0.02
    out = kernel(x=x, Wq=Wq, Wk=Wk, Wv=Wv, Wo=Wo)
    print("out", out.shape, out.dtype, float(np.abs(out).max()))


# revision 66
# speedup vs baseline: 1.0279x; 1.0252x over previous
"""GQA attention (B=2,T=2048,C=2048,H=16,KV=4,D=128) + RoPE + causal softmax
+ output projection, tensor-parallel over 8 NeuronCores (2 q-heads/core).

Contract: kernel(**inputs) takes full numpy inputs, returns full output.
Per-core partial outputs (o @ Wo[rows]) are summed on the host.
"""

import sys

sys.path.insert(0, "/opt/trn_rl_repo")

import numpy as np

import concourse.bacc as bacc
import concourse.bass as bass
import concourse.mybir as mybir
import concourse.tile as tile
from concourse.tile import add_dep_helper
from concourse.bass_utils import run_bass_kernel_spmd

B, T, C = 2, 2048, 2048
H, KV, D = 16, 4, 128
G = H // KV
N_CORES = 8
HL = H // N_CORES  # 2 q-heads per core
BT = B * T  # 4096
NCH = BT // 512  # 8 token chunks of 512
KCH = C // 128  # 16 contraction chunks
QC = T // 512  # 4 q chunks per batch
KC = T // 128  # 16 k chunks per batch

F16 = mybir.dt.float16
F32 = mybir.dt.float32

_cache = {}


class _St:
    pass


SPLIT_MM = False
USE_CC = True
DEBUG_KV = False
SS_GRP = 4  # score-tile group width (kc chunks per PSUM tile / exp call)
SS_BUFS = 1


def _mm2(nc, out, lhsT, rhs, start, stop):
    """M=128 matmul, optionally as two col-tiled M=64 halves (the second
    LDWEIGHTS can overlap the first matmul in the other column group)."""
    if not SPLIT_MM:
        nc.tensor.matmul(out[:, :], lhsT=lhsT, rhs=rhs,
                         start=start, stop=stop)
        return
    nc.tensor.matmul(out[0:64, :], lhsT=lhsT[:, 0:64], rhs=rhs,
                     start=start, stop=stop)
    nc.tensor.matmul(out[64:128, :], lhsT=lhsT[:, 64:128], rhs=rhs,
                     start=start, stop=stop, tile_position=(0, 64))


def _emit_p1_chunk(nc, st, n, with_kv=True):
    """QKV projections for one 512-token chunk.

    K/V are only computed for the core's own batch half (chunks 0..3);
    the partner core's half arrives via a 2-core AllGather."""
    mult = mybir.AluOpType.mult

    def rope(dst, ps, cos_s, sin_s, t0):
        t1 = st.rpool.tile([128, 512], F32, name="t1")
        t2 = st.rpool.tile([128, 512], F32, name="t2")
        nc.vector.tensor_tensor(t1[:], ps[:], cos_s[:, t0:t0 + 512], mult)
        nc.vector.tensor_tensor(t2[0:64, :], ps[64:128, :],
                                sin_s[0:64, t0:t0 + 512], mult)
        nc.vector.tensor_tensor(t2[64:128, :], ps[0:64, :],
                                sin_s[64:128, t0:t0 + 512], mult)
        nc.vector.tensor_tensor(dst, t1[:], t2[:], mybir.AluOpType.add)

    n0 = n * 512
    t0 = (n % QC) * 512  # rope-table offset (within batch)
    xt = st.xpool.tile([128, KCH, 512], F16, name="xt")
    dma_eng = nc.sync if n % 2 == 0 else nc.scalar
    dma_eng.dma_start(xt[:], st.xTp[n])
    for h in range(HL):
        psq = st.psA.tile([128, 512], F32, name="psq", tag="psA")
        for kc in range(KCH):
            _mm2(nc, psq, st.wq_s[:, kc, h * D:(h + 1) * D],
                 xt[:, kc, :], (kc == 0), (kc == KCH - 1))
        rope(st.qT[:, h, n0:n0 + 512], psq, st.cosq_s, st.sinq_s, t0)
    if not with_kv:
        return
    psk = st.psA.tile([128, 512], F32, name="psk", tag="psA")
    for kc in range(KCH):
        _mm2(nc, psk, st.wk_s[:, kc, :], xt[:, kc, :],
             (kc == 0), (kc == KCH - 1))
    rope(st.kT[:, n0:n0 + 512], psk, st.cosk_s, st.sink_s, t0)
    # v computed in transposed layout (N=512 matmuls), then moved to the
    # natural [token, d] layout via the DMA-transpose xbar (off-engine)
    psvt = st.psA.tile([128, 512], F32, name="psvt", tag="psA")
    for kc in range(KCH):
        _mm2(nc, psvt, st.wv_s[:, kc, :], xt[:, kc, :],
             (kc == 0), (kc == KCH - 1))
    vt = st.vtpool.tile([128, 512], F16, name="vt")
    nc.vector.tensor_copy(vt[:], psvt[:])
    # bounce through DRAM: the transpose xbar only supports writes from a
    # DRAM source reliably (SBUF->SBUF transpose wedges the device)
    nc.sync.dma_start(st.vtd[n], vt[:])
    for s in range(4):
        nc.sync.dma_start(st.vv[:, n * 4 + s, :],
                          st.vtd[n][:, s * 128:(s + 1) * 128], transpose=True)


def _emit_p2_qc(nc, st, b, h, qc):
    """Attention for one (batch, head, 512-wide q chunk)."""
    mult = mybir.AluOpType.mult
    base = b * T
    q0 = base + qc * 512
    nkc = 4 * (qc + 1)  # causal: k chunks 0..nkc-1
    grp = SS_GRP
    nquad = nkc // grp
    psv2 = st.psB.tile([128, 512], F32, name="psv2", tag="psB")
    psd = st.psB.tile([128, 512], F32, name="psd", tag="psB")
    pts = [None] * nquad

    def tile_off(kc):
        # diagonal tiles compute only the valid causal q-range
        return (kc - 4 * qc) * 128 if kc >= 4 * qc else 0

    def emit_scores(qi):
        pss = st.psS.tile([128, grp, 512], F32, name="pss")
        pt = st.ppool.tile([128, grp, 512], F16, name="pt")
        for i in range(grp):
            kc = qi * grp + i
            off = tile_off(kc)
            _mm2(nc, pss[:, i, off:512],
                 st.kT[:, base + kc * 128:base + (kc + 1) * 128],
                 st.qT[:, h, q0 + off:q0 + 512], True, True)
        nc.scalar.activation(pt[:], pss[:], mybir.ActivationFunctionType.Exp)
        for i in range(grp):
            kc = qi * grp + i
            off = tile_off(kc)
            if kc >= 4 * qc:  # mask: same pattern at every offset
                nc.vector.tensor_tensor(
                    pt[:, i, off:512], pt[:, i, off:512],
                    st.masks_s[:, 0, 0:512 - off], mult)
        pts[qi] = pt

    def emit_consume(qi):
        pt = pts[qi]
        for i in range(grp):
            kc = qi * grp + i
            off = tile_off(kc)
            first = (kc == 0)
            last = (kc == nkc - 1)
            nc.tensor.matmul(
                psd[0:1, off:512], lhsT=st.ones_s[:],
                rhs=pt[:, i, off:512], start=first, stop=last)
            _mm2(nc, psv2[:, off:512], st.vv[:, b * KC + kc, :],
                 pt[:, i, off:512], first, last)

    # 1-stage software pipeline: PE's consume matmuls never sit
    # immediately behind the exp they wait on
    for qi in range(nquad + 1):
        if qi < nquad:
            emit_scores(qi)
        if qi > 0:
            emit_consume(qi - 1)

    # evict both accumulators immediately so the psB ring never blocks the
    # next q-chunk; normalization happens out of PSUM
    oraw = st.opool.tile([128, 512], F16, name="oraw")
    nc.vector.tensor_copy(oraw[:], psv2[:])
    recip = st.rpool.tile([1, 512], F32, name="recip")
    nc.vector.reciprocal(recip[:], psd[0:1, :])
    bc = st.bpool.tile([128, 512], F32, name="bc")
    nc.gpsimd.partition_broadcast(bc[:], recip[:])
    nc.vector.tensor_tensor(st.oT[:, h, q0:q0 + 512], oraw[:], bc[:], mult)


def _emit_p3_group(nc, st, b, qc):
    """Output projection for the 4 token tiles covered by (b, qc)."""
    for i in range(4):
        t0 = b * T + qc * 512 + i * 128
        for cc in range(C // 512):
            c0 = cc * 512
            pso = st.psA.tile([128, 512], F32, name="pso", tag="psA")
            for h in range(HL):
                _mm2(nc, pso, st.oT[:, h, t0:t0 + 128],
                     st.wo_s[:, h, c0:c0 + 512], (h == 0), (h == HL - 1))
            yt = st.ypool.tile([128, 512], F16, name="yt")
            # balance PSUM eviction between ACT and DVE
            if st.p3_idx % 2 == 0:
                nc.scalar.copy(yt[:], pso[:])
            else:
                nc.vector.tensor_copy(yt[:], pso[:])
            dma_eng = nc.sync if st.p3_idx % 2 == 0 else nc.scalar
            st.p3_idx += 1
            dma_eng.dma_start(st.y[t0:t0 + 128, c0:c0 + 512], yt[:])


def _emit_cc(nc, st):
    """Pair K/V exchange: ship own half, receive partner half."""
    w1 = nc.sync.dma_start(st.kv_loc[:, 0:T], st.kT[:, 0:T])
    w2 = nc.sync.dma_start(
        st.kv_loc[:, T:2 * T],
        st.vv[:, 0:KC, :].rearrange("p a b -> p (a b)"))
    cc = nc.gpsimd.collective_compute(
        "AllGather",
        mybir.AluOpType.bypass,
        replica_groups=st.replica_groups,
        ins=[st.kv_loc[:]],
        outs=[st.kv_all[:]],
    )
    add_dep_helper(cc.ins, w1.ins, reason="cc after kv_loc k write")
    add_dep_helper(cc.ins, w2.ins, reason="cc after kv_loc v write")
    # partner slot in the gathered buffer is rank-relative: 1 - (pid & 1)
    partner = 1 - (nc.partition_id() & 1)
    prow = partner * 128
    r1 = nc.sync.dma_start(st.kT[:, T:2 * T],
                           st.kv_all[bass.ds(prow, 128), 0:T])
    r2 = nc.sync.dma_start(
        st.vv[:, KC:2 * KC, :].rearrange("p a b -> p (a b)"),
        st.kv_all[bass.ds(prow, 128), T:2 * T])
    add_dep_helper(r1.ins, cc.ins, reason="k read after cc")
    add_dep_helper(r2.ins, cc.ins, reason="v read after cc")


def _emit_body(nc, st, emit_cc=True):
    """One full forward pass, phases interleaved so ACT's exp work overlaps
    PE-heavy projection / output-projection stretches.

    Batch slots are core-local: slot 0 is this core's own batch (K/V
    computed locally), slot 1 is the partner core's batch (K/V arrive via
    the pair AllGather). The host un-permutes odd cores' outputs."""
    st.p3_idx = 0
    for n in range(QC):  # own-batch token chunks: full q/k/v
        _emit_p1_chunk(nc, st, n, with_kv=True)
    if st.use_cc and emit_cc:
        _emit_cc(nc, st)
    for qc in range(QC):  # slot-0/h0 attention overlaps slot-1 projections
        _emit_p2_qc(nc, st, 0, 0, qc)
        _emit_p1_chunk(nc, st, QC + qc, with_kv=not st.use_cc)
    for qc in range(QC):  # slot-0/h1 attention overlaps slot-0 out-proj
        _emit_p2_qc(nc, st, 0, 1, qc)
        _emit_p3_group(nc, st, 0, qc)
    for qc in range(QC):  # slot-1 attention (both heads) + slot-1 out-proj
        _emit_p2_qc(nc, st, 1, 0, qc)
        _emit_p2_qc(nc, st, 1, 1, qc)
        _emit_p3_group(nc, st, 1, qc)
    if st.debug_kv:
        nc.sync.dma_start(st.kt_dbg[:], st.kT[:])
        nc.sync.dma_start(st.vv_dbg[:],
                          st.vv[:].rearrange("p a b -> p (a b)"))


def _build_program(reps=1, loop_n=None):
    nc = bacc.Bacc("TRN2", target_bir_lowering=False, debug=False,
                   num_devices=N_CORES)

    # all inputs arrive pre-packed so every DMA is a contiguous burst
    xTp = nc.dram_tensor("xTp", [NCH, 128, KCH, 512], F16,
                         kind="ExternalInput").ap()
    wq = nc.dram_tensor("wq", [128, KCH, HL * D], F16,
                        kind="ExternalInput").ap()
    wk = nc.dram_tensor("wk", [128, KCH, D], F16, kind="ExternalInput").ap()
    wv = nc.dram_tensor("wv", [128, KCH, D], F16, kind="ExternalInput").ap()
    wo = nc.dram_tensor("wo", [128, HL, C], F16, kind="ExternalInput").ap()
    cosq = nc.dram_tensor("cosq", [D, T], F16, kind="ExternalInput").ap()
    sinq = nc.dram_tensor("sinq", [D, T], F16, kind="ExternalInput").ap()
    cosk = nc.dram_tensor("cosk", [D, T], F16, kind="ExternalInput").ap()
    sink = nc.dram_tensor("sink", [D, T], F16, kind="ExternalInput").ap()
    masks = nc.dram_tensor("masks", [128, 4, 512], F16, kind="ExternalInput").ap()
    y = nc.dram_tensor("y", [BT, C], F16, kind="ExternalOutput").ap()
    vtd = nc.dram_tensor("vtd", [NCH, D, 512], F16).ap()
    kv_loc = nc.dram_tensor("kv_loc", [D, 2 * T], F16).ap()
    kv_all = nc.dram_tensor("kv_all", [2 * D, 2 * T], F16).ap()
    if DEBUG_KV:
        kt_dbg = nc.dram_tensor("kt_dbg", [D, BT], F16,
                                kind="ExternalOutput").ap()
        vv_dbg = nc.dram_tensor("vv_dbg", [128, BT], F16,
                                kind="ExternalOutput").ap()

    st = _St()
    with tile.TileContext(nc) as tc:
        with (
            tc.tile_pool(name="cpool", bufs=1) as cpool,
            tc.tile_pool(name="xpool", bufs=3) as xpool,
            tc.tile_pool(name="ppool", bufs=6) as ppool,
            tc.tile_pool(name="rpool", bufs=3) as rpool,
            tc.tile_pool(name="bpool", bufs=2) as bpool,
            tc.tile_pool(name="opool", bufs=2) as opool,
            tc.tile_pool(name="vtpool", bufs=2) as vtpool,
            tc.tile_pool(name="ypool", bufs=10) as ypool,
            tc.tile_pool(name="psA", bufs=2, space="PSUM") as psA,
            tc.tile_pool(name="psB", bufs=2, space="PSUM") as psB,
            tc.tile_pool(name="psS", bufs=SS_BUFS, space="PSUM") as psS,
        ):
            st.xpool, st.ppool, st.rpool, st.bpool, st.ypool = (
                xpool, ppool, rpool, bpool, ypool)
            st.opool = opool
            st.vtpool = vtpool
            st.psA, st.psB, st.psS = psA, psB, psS

            # ---- persistent SBUF state ----
            st.wq_s = cpool.tile([128, KCH, HL * D], F16, name="wq_s")
            st.wk_s = cpool.tile([128, KCH, D], F16, name="wk_s")
            st.wv_s = cpool.tile([128, KCH, D], F16, name="wv_s")
            st.wo_s = cpool.tile([128, HL, C], F16, name="wo_s")
            st.cosq_s = cpool.tile([D, T], F16, name="cosq_s")
            st.sinq_s = cpool.tile([D, T], F16, name="sinq_s")
            st.cosk_s = cpool.tile([D, T], F16, name="cosk_s")
            st.sink_s = cpool.tile([D, T], F16, name="sink_s")
            st.masks_s = cpool.tile([128, 4, 512], F16, name="masks_s")
            st.ones_s = cpool.tile([128, 1], F16, name="ones_s")
            st.qT = cpool.tile([D, HL, BT], F16, name="qT")
            st.kT = cpool.tile([D, BT], F16, name="kT")
            st.vv = cpool.tile([128, BT // 128, D], F16, name="vv")
            st.oT = cpool.tile([D, HL, BT], F16, name="oT")

            # ordered by first-use time so the first matmul starts ASAP:
            # wq+xt0 gate matmul #1, rope tables gate the first evictions,
            # masks/wo are not needed until attention / output projection
            nc.sync.dma_start(st.wq_s[:], wq[:])
            nc.scalar.dma_start(st.cosq_s[:], cosq[:])
            nc.scalar.dma_start(st.sinq_s[:], sinq[:])
            nc.scalar.dma_start(st.wk_s[:], wk[:])
            nc.scalar.dma_start(st.cosk_s[:], cosk[:])
            nc.scalar.dma_start(st.sink_s[:], sink[:])
            nc.scalar.dma_start(st.wv_s[:], wv[:])
            nc.scalar.dma_start(st.masks_s[:], masks[:])
            nc.scalar.dma_start(st.wo_s[:], wo[:])
            nc.vector.memset(st.ones_s[:], 1.0)

            st.xTp = xTp
            st.y = y
            st.vtd = vtd
            st.kv_loc = kv_loc
            st.kv_all = kv_all
            st.use_cc = USE_CC
            st.replica_groups = [[2 * i, 2 * i + 1]
                                 for i in range(N_CORES // 2)]
            st.debug_kv = DEBUG_KV
            if DEBUG_KV:
                st.kt_dbg, st.vv_dbg = kt_dbg, vv_dbg

            if loop_n is not None:
                engs = (mybir.EngineType.PE, mybir.EngineType.Activation,
                        mybir.EngineType.DVE, mybir.EngineType.SP,
                        mybir.EngineType.Pool)
                if USE_CC:
                    # collectives desync inside For_i; run the exchange once
                    # before the loop (its cost hides behind q projections)
                    _emit_cc(nc, st)
                with tc.For_i(0, loop_n, 1, hint_engines=engs):
                    _emit_body(nc, st, emit_cc=False)
            else:
                for _rep in range(reps):
                    _emit_body(nc, st)

    nc.compile()
    return nc


def _rope_tables():
    inv = (1.0 / (10000.0 ** (np.arange(0, D, 2, dtype=np.float32) / D)))
    f = np.arange(T, dtype=np.float32)[:, None] * inv[None, :]  # [T, 64]
    cos = np.concatenate([np.cos(f)] * 2, axis=-1).astype(np.float32)  # [T,128]
    sin = np.concatenate([np.sin(f)] * 2, axis=-1).astype(np.float32)
    sgn = np.ones((D,), np.float32)
    sgn[0:64] = -1.0  # sign-folded for the rotate-half formulation
    sinf = sin * sgn[None, :]
    alpha = np.float32(1.0 / np.sqrt(D))
    return (cos.T.copy() * alpha, sinf.T.copy() * alpha,
            cos.T.copy(), sinf.T.copy())


def _masks():
    i = np.arange(128)[:, None]
    j = np.arange(512)[None, :]
    m = np.stack([(j >= i + 128 * d) for d in range(4)], axis=0)  # [4,128,512]
    return np.ascontiguousarray(m.transpose(1, 0, 2)).astype(np.float16)


def _pack_w(w):
    """[K*128, M] -> [128, K, M] (per-partition contiguous)."""
    kch, m = w.shape[0] // 128, w.shape[1]
    return np.ascontiguousarray(
        w.reshape(kch, 128, m).transpose(1, 0, 2)).astype(np.float16)


def make_in_maps(x, Wq, Wk, Wv, Wo):
    # xTp[n, p, k, j] = x^T[k*128+p, n*512+j]
    xT = x.reshape(BT, C).T.astype(np.float16)  # [C, BT]
    xTp = np.ascontiguousarray(
        xT.reshape(KCH, 128, NCH, 512).transpose(2, 1, 0, 3))
    # under the CC scheme, odd cores see their own batch (b1) in slot 0
    xTp_odd = (np.ascontiguousarray(np.roll(xTp, QC, axis=0))
               if USE_CC else xTp)
    cq, sq, ck, sk = [t.astype(np.float16) for t in _rope_tables()]
    mk = _masks()
    in_maps = []
    for c in range(N_CORES):
        g = c // 2  # kv head for this core's 2 q-heads
        in_maps.append({
            "xTp": xTp if c % 2 == 0 else xTp_odd,
            "wq": _pack_w(Wq[:, c * HL * D:(c + 1) * HL * D]),
            "wk": _pack_w(Wk[:, g * D:(g + 1) * D]),
            "wv": _pack_w(Wv[:, g * D:(g + 1) * D]),
            "wo": _pack_w(Wo[c * HL * D:(c + 1) * HL * D, :]),
            "cosq": cq, "sinq": sq, "cosk": ck, "sink": sk,
            "masks": mk,
        })
    return in_maps


def get_program(reps=1, loop_n=None):
    key = ("nc", reps, loop_n)
    if key not in _cache:
        _cache[key] = _build_program(reps, loop_n)
    return _cache[key]


def kernel(x, Wq, Wk, Wv, Wo):
    # coerce to host numpy up front (harness may pass jax arrays)
    x, Wq, Wk, Wv, Wo = (np.asarray(a, dtype=np.float32)
                         for a in (x, Wq, Wk, Wv, Wo))
    nc = get_program()
    in_maps = make_in_maps(x, Wq, Wk, Wv, Wo)
    res = run_bass_kernel_spmd(nc, in_maps, core_ids=list(range(N_CORES)))
    out = np.zeros((BT, C), np.float32)
    for c in range(N_CORES):
        yc = res.results[c]["y"].astype(np.float32)
        if USE_CC and c % 2 == 1:  # un-permute batch slots
            yc = np.concatenate([yc[T:], yc[:T]], axis=0)
        out += yc
    return out.reshape(B, T, C)


if __name__ == "__main__":
    rng = np.random.default_rng(0)
    x = rng.standard_normal((B, T, C), dtype=np.float32)
    Wq = rng.standard_normal((C, H * D), dtype=np.float32) * 0.02
    Wk = rng.standard_normal((C, KV * D), dtype=np.float32) * 0.02
    Wv = rng.standard_normal((C, KV * D), dtype=np.float32) * 0.02
    Wo = rng.standard_normal((C, C), dtype=np.float32) * 0.02
    out = kernel(x=x, Wq=Wq, Wk=Wk, Wv=Wv, Wo=Wo)
    print("out", out.shape, out.dtype, float(np.abs(out).max()))
